# revision 1
# baseline (speedup 1.0000x reference)
"""Multi-head attention with RoPE on 8 Trainium2 NeuronCores.

Sharding: data-parallel over batch (2) x tensor-parallel over heads (4 groups
of 4 heads).  Core g handles batch g//4, heads 4*(g%4) .. 4*(g%4)+4.

Per-core device program (all matmuls f32r):
  - Phase 1: Q/K/V projections from host-pre-transposed x^T [1024, T]
    inputs, streaming the 8 contraction chunks through SBUF while the
    accumulation groups live in PSUM banks (8 at a time).  W_q/W_k rows are
    host-permuted so each head's channels come out deinterleaved
    ([evens; odds]), turning interleaved RoPE into rotate-half RoPE on
    contiguous 32-row blocks (S = Q.K is invariant to a shared channel
    permutation of Q and K).  RoPE runs on DVE straight out of PSUM with
    host-precomputed cos/sin tables.
  - Phase 2: attention in transposed layout, Tq blocks of 1024 handled as
    two 512 halves (f32r moving-operand limit) sharing one PSUM tile and
    one exp: S^T[Tk-chunk, Tq] = K @ Q^T per head (Q^T padded to 128
    contraction rows with zeros), exp on ACT with the 1/sqrt(dk) scale
    fused (max |S| ~ 9 so softmax without max-subtraction is safe in fp32),
    P^T V accumulated over Tk chunks with a ones column appended to V so
    the softmax denominator falls out of the same matmuls.  Normalization
    via PE outer-product broadcast of the reciprocal row.  W_o row-parallel
    partial product -> y^T partial [1024, T] per core.
Host sums the 4 partials of each batch and transposes back.
"""

import numpy as np

import concourse.bass as bass
import concourse.mybir as mybir
import concourse.tile as tile
from concourse import bacc
from concourse import bass_utils
from contextlib import ExitStack

P = 128
D_MODEL = 1024
N_HEADS = 16
DK = 64
T = 2048
B = 2
ROPE_BASE = 10000.0
GH = 4          # heads per core
DH = GH * DK    # channels per core (256)
KC = D_MODEL // P   # 8 contraction chunks
TBLK = 512
NBLK = T // TBLK    # 4
TB2 = 1024
NB2 = T // TB2      # 2
NTC = T // P        # 16 Tk chunks
F32 = mybir.dt.float32
F32R = mybir.dt.float32r
BF16 = mybir.dt.bfloat16
EXP = mybir.ActivationFunctionType.Exp
# When True, the x inputs and W_q/W_k/W_v weights are shipped and consumed
# in bfloat16: halves the input-DMA/upload volume at ~1e-3 relative error.
BF16_INPUTS = False
XDT = BF16 if BF16_INPUTS else F32R


def emit(nc, io, reps=1):
    with ExitStack() as ctx:
        ctx.enter_context(nc.allow_low_precision(
            reason="f32r rounding of matmul operands is intentional"))
        tc = ctx.enter_context(tile.TileContext(nc))
        const = ctx.enter_context(tc.tile_pool(name="const", bufs=1))
        persist = ctx.enter_context(tc.tile_pool(name="persist", bufs=1))
        rsc = ctx.enter_context(tc.tile_pool(name="ropescr", bufs=2))
        esp = ctx.enter_context(tc.tile_pool(name="esp", bufs=3))
        otp = ctx.enter_context(tc.tile_pool(name="otp", bufs=2))
        ysp = ctx.enter_context(tc.tile_pool(name="ysp", bufs=2))
        rcp = ctx.enter_context(tc.tile_pool(name="rcp", bufs=1))
        bsp = ctx.enter_context(tc.tile_pool(name="bsp", bufs=1))

        # ---- persistent activation storage ----
        # Qpad[h][blk]: [128, TBLK]; head data at rows (h%2)*64, rest zero.
        qpad = [[persist.tile([P, TBLK], F32R, tag=f"qp{h}_{b}",
                              name=f"qp{h}_{b}") for b in range(NBLK)]
                for h in range(GH)]
        for h in range(GH):
            off = (1 - h % 2) * DK
            for b in range(NBLK):
                nc.gpsimd.memset(qpad[h][b][off:off + DK, :].bitcast(F32), 0.0)
        # Kr[u][blk]: roped K^T for heads 2u,2u+1
        kr = [[persist.tile([P, TBLK], F32R, tag=f"kr{u}_{b}",
                            name=f"kr{u}_{b}") for b in range(NBLK)]
              for u in range(2)]
        # V[c]: [128, 4, 65] (per head 64 cols + ones col)
        vt = [persist.tile([P, GH, DK + 1], F32R, tag=f"v{c}", name=f"v{c}")
              for c in range(NTC)]
        for c in range(NTC):
            nc.gpsimd.memset(vt[c][:, :, DK].bitcast(F32), 1.0)

        # ---- constants (weight chunk DMAs are emitted inline with the x
        # streams so the first matmuls are not stuck behind bulk loads) ----
        wq_t = const.tile([P, KC, DH], XDT, tag="wq", name="wq")
        wk_t = const.tile([P, KC, DH], XDT, tag="wk", name="wk")
        wv_t = const.tile([P, KC, DH], XDT, tag="wv", name="wv")
        cos_t = const.tile([P, T], F32, tag="cos", name="cos")
        sin_t = const.tile([P, T], F32, tag="sin", name="sin")
        wo_t = const.tile([P, 2, D_MODEL], F32R, tag="wo", name="wo")
        e0 = const.tile([P, DK], F32R, tag="e0", name="e0")
        nc.gpsimd.memset(e0[:].bitcast(F32), 0.0)
        nc.gpsimd.memset(e0[0:1, :].bitcast(F32), 1.0)
        swm = const.tile([P, P], F32R, tag="swm", name="swm")
        nc.scalar.dma_start(swm[:], io["swapM"][:])
        wmap = {"xkT": ("wkT", wk_t), "xqT": ("wqT", wq_t),
                "xvT": ("wvT", wv_t)}

        def rope_from_psum(ps, oc, blk, dest_of_head, vs_alloc):
            """dest rows get rotate-half rope of psum proj tile.

            HW requires SBUF+SBUF tensor-op inputs to share a base
            partition, so the cross-half sin product is partition-swapped
            through the PE (constant permutation matmul into a recycled
            PSUM slot); the combining ops then read SBUF+PSUM pairs.
            """
            u = rsc.tile([P, TBLK], F32, tag="t1", name="u")
            v = rsc.tile([P, TBLK], F32R, tag="t2", name="v")
            cb = cos_t[:, blk * TBLK:(blk + 1) * TBLK]
            sb = sin_t[:, blk * TBLK:(blk + 1) * TBLK]
            nc.vector.tensor_mul(out=u[:], in0=ps[:], in1=cb)
            nc.vector.tensor_mul(out=v[:], in0=ps[:], in1=sb)
            vs = vs_alloc()
            nc.tensor.matmul(vs[:], lhsT=swm[:], rhs=v[:],
                             start=True, stop=True)
            for hl in range(2):
                h = oc * 2 + hl
                dst, base = dest_of_head(h)
                x1 = slice(hl * DK, hl * DK + 32)
                x2 = slice(hl * DK + 32, hl * DK + DK)
                nc.vector.tensor_sub(out=dst[base:base + 32, :],
                                     in0=u[x1, :], in1=vs[x1, :])
                nc.vector.tensor_add(out=dst[base + 32:base + DK, :],
                                     in0=u[x2, :], in1=vs[x2, :])

        for rep in range(reps):
            # ---- phase 1: K, V, then Q projections (PSUM accumulators) ----
            xbig_ctx = ExitStack()
            xbig = xbig_ctx.enter_context(tc.tile_pool(name=f"xbig{rep}", bufs=3))
            with tc.tile_pool(name=f"ps1_{rep}", bufs=8, space="PSUM") as ps1:
                # K: 8 psum accumulators [oc][blk], stream xk chunks.
                kps = {(oc, blk): ps1.tile([P, TBLK], F32, tag="ph1",
                                           name=f"kps{oc}_{blk}")
                       for oc in range(2) for blk in range(NBLK)}
                for kc in range(KC):
                    nc.scalar.dma_start(wk_t[:, kc, :],
                                        io["wkT"][kc * P:(kc + 1) * P, :])
                    eng = nc.sync if kc % 2 == 0 else nc.scalar
                    xt = xbig.tile([P, T], XDT, tag="x", name="xt")
                    eng.dma_start(xt[:], io["xkT"][kc * P:(kc + 1) * P, :])
                    if kc == 0:
                        nc.scalar.dma_start(cos_t[:], io["cosT"][:])
                        nc.scalar.dma_start(sin_t[:], io["sinT"][:])
                    for oc in range(2):
                        for blk in range(NBLK):
                            nc.tensor.matmul(
                                kps[(oc, blk)][:],
                                lhsT=wk_t[:, kc, oc * P:(oc + 1) * P],
                                rhs=xt[:, blk * TBLK:(blk + 1) * TBLK],
                                start=(kc == 0), stop=(kc == KC - 1))
                # wo not needed until phase 2 -- load behind the K stream
                nc.scalar.dma_start(
                    wo_t[:], io["woT"].rearrange("(o p) f -> p o f", p=P))
                for oc in range(2):
                    for blk in range(NBLK):
                        rope_from_psum(
                            kps[(oc, blk)], oc, blk,
                            lambda h, oc=oc, blk=blk: (kr[oc][blk],
                                                       (h % 2) * DK),
                            lambda: ps1.tile([P, TBLK], F32, tag="ph1",
                                             name="vs_ps"))

                # V projection in two waves of 8 Tk chunks; each wave streams the
                # matching column-half of xv and holds 8 PSUM accumulators.
                for w in range(2):
                    vps = [ps1.tile([P, DH], F32, tag="ph1", name=f"vps{w}_{i}")
                           for i in range(8)]
                    for kc in range(KC):
                        if w == 0:
                            nc.scalar.dma_start(wv_t[:, kc, :],
                                                io["wvT"][kc * P:(kc + 1) * P, :])
                        eng = nc.sync if kc % 2 == 0 else nc.scalar
                        xt = xbig.tile([P, T // 2], XDT, tag="x", name="xv")
                        eng.dma_start(
                            xt[:], io["xvT"][kc * P:(kc + 1) * P,
                                             w * (T // 2):(w + 1) * (T // 2)])
                        for cl in range(8):
                            nc.tensor.matmul(
                                vps[cl][:],
                                lhsT=xt[:, cl * P:(cl + 1) * P],
                                rhs=wv_t[:, kc, :],
                                start=(kc == 0), stop=(kc == KC - 1))
                    for cl in range(8):
                        c = w * 8 + cl
                        nc.vector.tensor_copy(
                            out=vt[c][:, :, 0:DK],
                            in_=vps[cl].rearrange("p (h d) -> p h d", h=GH))

            # psA coexists with Q projection: q(2) + s(4) + o(2) = 8 banks, so
            # attention can start while Q blocks 2-3 are still projecting.
            ps2_ctx = ExitStack()
            ps2 = ps2_ctx.enter_context(tc.tile_pool(name=f"ps2_{rep}",
                                                     bufs=1, space="PSUM"))

            # Q: block-major so each block's rope runs while the next block
            # streams, letting attention start as soon as blocks 0-1 land.
            for kc in range(KC):
                nc.scalar.dma_start(wq_t[:, kc, :],
                                    io["wqT"][kc * P:(kc + 1) * P, :])
            for blk in range(NBLK):
                qps = [ps2.tile([P, TBLK], F32, tag="q", bufs=2,
                                name=f"qps{oc}") for oc in range(2)]
                for kc in range(KC):
                    eng = nc.sync if kc % 2 == 0 else nc.scalar
                    xt = xbig.tile([P, TBLK], XDT, tag="xq", name="xq")
                    eng.dma_start(
                        xt[:], io["xqT"][kc * P:(kc + 1) * P,
                                         blk * TBLK:(blk + 1) * TBLK])
                    for oc in range(2):
                        nc.tensor.matmul(
                            qps[oc][:],
                            lhsT=wq_t[:, kc, oc * P:(oc + 1) * P],
                            rhs=xt[:],
                            start=(kc == 0), stop=(kc == KC - 1))
                for oc in range(2):
                    rope_from_psum(
                        qps[oc], oc, blk,
                        lambda h, blk=blk: (qpad[h][blk], (h % 2) * DK),
                        lambda: ps2.tile([P, TBLK], F32, tag="q", bufs=2,
                                         name="vs_ps"))
            xbig_ctx.close()

            # ---- phase 2: attention + W_o per Tq-1024 block ----
            if True:
                for b2 in range(NB2):
                    ot = [otp.tile([P, TB2], F32R, tag=f"ot{u}", name=f"ot{u}")
                          for u in range(2)]
                    for h in range(GH):
                        ops = ps2.tile([DK + 1, TB2], F32, tag="o", bufs=1,
                                       name="ops")
                        for c in range(NTC):
                            sp = ps2.tile([P, TB2], F32, tag="s", bufs=2,
                                          name="sp")
                            for hf in range(2):
                                blk = b2 * 2 + hf
                                nc.tensor.matmul(
                                    sp[:, hf * TBLK:(hf + 1) * TBLK],
                                    lhsT=kr[h // 2][c // 4][:, (c % 4) * P:
                                                            (c % 4 + 1) * P],
                                    rhs=qpad[h][blk][:],
                                    start=True, stop=True)
                            es = esp.tile([P, TB2], F32R, tag="es", name="es")
                            nc.scalar.activation(es[:], sp[:], EXP, scale=0.125)
                            for hf in range(2):
                                nc.tensor.matmul(
                                    ops[:, hf * TBLK:(hf + 1) * TBLK],
                                    lhsT=vt[c][:, h, :],
                                    rhs=es[:, hf * TBLK:(hf + 1) * TBLK],
                                    start=(c == 0), stop=(c == NTC - 1))
                        # normalize: rows 0..63 / row 64
                        rt = rcp.tile([P, TB2], F32R, tag="rt", name="rt")
                        nc.gpsimd.memset(rt[:].bitcast(F32), 0.0)
                        nc.vector.reciprocal(rt[0:1, :], ops[DK:DK + 1, :])
                        bs = bsp.tile([DK, TB2], F32, tag="bs", name="bs")
                        for hf in range(2):
                            bpt = ps2.tile([P, TBLK], F32, tag="q", bufs=2,
                                           name="bpt")
                            nc.tensor.matmul(
                                bpt[0:DK, :],
                                lhsT=e0[:],
                                rhs=rt[:, hf * TBLK:(hf + 1) * TBLK],
                                start=True, stop=True)
                            nc.vector.tensor_copy(
                                out=bs[:, hf * TBLK:(hf + 1) * TBLK],
                                in_=bpt[0:DK, :])
                        base = (h % 2) * DK
                        nc.vector.tensor_mul(out=ot[h // 2][base:base + DK, :],
                                             in0=ops[0:DK, :], in1=bs[:])

                    # W_o partial: y^T[i*128.., b2] = sum_u woT_chunk.T @ ot[u]
                    for i in range(KC):
                        for hf in range(2):
                            yp = ps2.tile([P, TBLK], F32, tag="q", bufs=2,
                                          name="yp")
                            for u in range(2):
                                nc.tensor.matmul(
                                    yp[:],
                                    lhsT=wo_t[:, u, i * P:(i + 1) * P],
                                    rhs=ot[u][:, hf * TBLK:(hf + 1) * TBLK],
                                    start=(u == 0), stop=(u == 1))
                            ys = ysp.tile([P, TBLK], F32, tag="ys", name="ys")
                            nc.vector.tensor_copy(out=ys[:], in_=yp[:])
                            nc.sync.dma_start(
                                io["ypT"][i * P:(i + 1) * P,
                                          (b2 * 2 + hf) * TBLK:
                                          (b2 * 2 + hf + 1) * TBLK],
                                ys[:])
            ps2_ctx.close()


def build_program(reps=1):
    nc = bacc.Bacc("TRN2", target_bir_lowering=False, debug=False,
                   num_devices=8)
    io = {}
    for name in ("xqT", "xkT", "xvT"):
        io[name] = nc.dram_tensor(name, [D_MODEL, T], XDT,
                                  kind="ExternalInput").ap()
    for name in ("wqT", "wkT", "wvT"):
        io[name] = nc.dram_tensor(name, [D_MODEL, DH], XDT,
                                  kind="ExternalInput").ap()
    io["woT"] = nc.dram_tensor("woT", [DH, D_MODEL], F32R,
                               kind="ExternalInput").ap()
    io["swapM"] = nc.dram_tensor("swapM", [P, P], F32R,
                                 kind="ExternalInput").ap()
    io["cosT"] = nc.dram_tensor("cosT", [P, T], F32,
                                kind="ExternalInput").ap()
    io["sinT"] = nc.dram_tensor("sinT", [P, T], F32,
                                kind="ExternalInput").ap()
    io["ypT"] = nc.dram_tensor("ypT", [D_MODEL, T], F32,
                               kind="ExternalOutput").ap()
    emit(nc, io, reps=reps)
    nc.compile()
    return nc


_PERM = np.concatenate(
    [h * DK + np.r_[np.arange(0, DK, 2), np.arange(1, DK, 2)]
     for h in range(N_HEADS)])


def rope_tables():
    # row j of a [128, T] tile <-> frequency index j % 32
    inv = 1.0 / (ROPE_BASE ** (np.arange(0, DK, 2, dtype=np.float32) / DK))
    pos = np.arange(T, dtype=np.float32)
    fr = np.outer(inv, pos)  # [32, T]
    fr = np.tile(fr, (4, 1))  # [128, T]
    return np.cos(fr).astype(np.float32), np.sin(fr).astype(np.float32)


def make_in_maps(q, k, v, W_q, W_k, W_v, W_o):
    import ml_dtypes
    xdt = ml_dtypes.bfloat16 if BF16_INPUTS else np.float32
    q = np.asarray(q, np.float32)
    k = np.asarray(k, np.float32)
    v = np.asarray(v, np.float32)
    Wq = np.asarray(W_q, np.float32)[_PERM].astype(xdt)
    Wk = np.asarray(W_k, np.float32)[_PERM].astype(xdt)
    Wv = np.asarray(W_v, np.float32).astype(xdt)
    Wo = np.asarray(W_o, np.float32)
    cos, sin = rope_tables()
    swm = np.zeros((P, P), np.float32)
    swm[np.arange(P), np.arange(P) ^ 32] = 1.0
    xT = {}
    for b in range(B):
        xT[b] = (np.ascontiguousarray(q[b].T).astype(xdt),
                 np.ascontiguousarray(k[b].T).astype(xdt),
                 np.ascontiguousarray(v[b].T).astype(xdt))
    in_maps = []
    for core in range(8):
        b, g = core // 4, core % 4
        cs = slice(g * DH, (g + 1) * DH)
        in_maps.append({
            "xqT": xT[b][0], "xkT": xT[b][1], "xvT": xT[b][2],
            "wqT": np.ascontiguousarray(Wq[cs].T),
            "wkT": np.ascontiguousarray(Wk[cs].T),
            "wvT": np.ascontiguousarray(Wv[cs].T),
            "woT": np.ascontiguousarray(Wo[:, cs].T),
            "cosT": cos, "sinT": sin, "swapM": swm,
        })
    return in_maps


_CACHE = {}


def _build_runner(nc):
    """One-time jitted SPMD executable over 8 cores.

    Mirrors bass_utils.run_bass_kernel_spmd's axon path
    (bass2jax.run_bass_via_pjrt) but caches the shard_map jit so repeated
    kernel() calls skip retracing/recompiling.
    """
    import jax
    from jax.sharding import Mesh, PartitionSpec
    from jax.experimental.shard_map import shard_map
    import concourse.mybir as mybir_
    from concourse import bass2jax

    bass2jax.install_neuronx_cc_hook()
    part_name = (nc.partition_id_tensor.name
                 if nc.partition_id_tensor else None)
    in_names, out_names, out_avals = [], [], []
    for alloc in nc.m.functions[0].allocations:
        if not isinstance(alloc, mybir_.MemoryLocationSet):
            continue
        name = alloc.memorylocations[0].name
        if alloc.kind == "ExternalInput":
            if name != part_name:
                in_names.append(name)
        elif alloc.kind == "ExternalOutput":
            out_names.append(name)
            out_avals.append(jax.core.ShapedArray(
                tuple(alloc.tensor_shape), mybir_.dt.np(alloc.dtype)))
    n_params = len(in_names)
    all_names = in_names + out_names
    if part_name is not None:
        all_names = all_names + [part_name]

    def _body(*args):
        operands = list(args)
        if part_name is not None:
            operands.append(bass2jax.partition_id_tensor())
        outs = bass2jax._bass_exec_p.bind(
            *operands, out_avals=tuple(out_avals), in_names=tuple(all_names),
            out_names=tuple(out_names), lowering_input_output_aliases=(),
            sim_require_finite=True, sim_require_nnan=True, nc=nc)
        return tuple(outs)

    devices = jax.devices()[:8]
    mesh = Mesh(np.asarray(devices), ("core",))
    n_outs = len(out_names)
    sharded = jax.jit(
        shard_map(_body, mesh=mesh,
                  in_specs=(PartitionSpec("core"),) * (n_params + n_outs),
                  out_specs=(PartitionSpec("core"),) * n_outs,
                  check_rep=False),
        keep_unused=True)
    from jax.sharding import NamedSharding
    shard = NamedSharding(mesh, PartitionSpec("core"))
    zero_outs = [jax.device_put(
        np.zeros((8 * a.shape[0], *a.shape[1:]), a.dtype), shard)
        for a in out_avals]
    return sharded, in_names, out_names, out_avals, zero_outs


def _run_spmd(in_maps):
    nc = _CACHE["nc"]
    if "runner" not in _CACHE:
        _CACHE["runner"] = _build_runner(nc)
    sharded, in_names, out_names, out_avals, zero_outs = _CACHE["runner"]
    concat_in = [np.concatenate([np.asarray(in_maps[c][n])
                                 for c in range(8)], axis=0)
                 for n in in_names]
    out_arrs = sharded(*concat_in, *zero_outs)
    return [{n: np.asarray(out_arrs[i]).reshape(8, *out_avals[i].shape)[c]
             for i, n in enumerate(out_names)} for c in range(8)]


def kernel(q, k, v, W_q, W_k, W_v, W_o):
    if "nc" not in _CACHE:
        _CACHE["nc"] = build_program()
    in_maps = make_in_maps(q, k, v, W_q, W_k, W_v, W_o)
    try:
        results = _run_spmd(in_maps)
    except Exception:
        # fall back to the stock runner (fresh jit per call, slower wall
        # clock but the same device program)
        _CACHE.pop("runner", None)
        res = bass_utils.run_bass_kernel_spmd(
            _CACHE["nc"], in_maps, core_ids=list(range(8)))
        results = res.results
    out = np.empty((B, T, D_MODEL), np.float32)
    for b in range(B):
        acc = results[b * 4]["ypT"].astype(np.float32).copy()
        for g in range(1, 4):
            acc += results[b * 4 + g]["ypT"]
        out[b] = acc.T
    return out

